# revision 19
# baseline (speedup 1.0000x reference)
"""Multi-head causal attention (B=2, S=2048, D=1024, H=16) on 8 TRN2 NeuronCores.

Sharding: batch*head parallel. Core c handles batch b = c//4 and the 4
heads h in [4*(c%4), 4*(c%4)+4). Each core computes its heads' Q/K/V
projections (column-parallel), causal softmax attention, and its partial
row-parallel output projection; the host sums the 4 partial outputs per
batch (the AllReduce of row-parallel tensor parallelism).

On-device layout: everything is kept "transposed" (feature-major) so
every matmul contracts along the partition dimension:
  scoresT[k,q] = K Q^T      (per head, 128-row k-tiles x 512-col q-tiles)
  P^T = exp(scoresT/8 + mask/8)   (additive -1e9 causal mask)
  outT[d,q]   = sum_k V[k,d] P^T[k,q]   (PSUM-accumulated over k-tiles)
  sums[q]     = sum_k P^T[k,q]          (ones-vector matmul, col-packed)
  y[q,e]     += sum_hd outT_norm[hd,q] * w_oT[hd,e]
Softmax skips the max-subtraction: scores ~ N(0,1) so exp never
overflows, and exp(-1e9/8) underflows to exactly 0 like the
reference's masked_fill(-1e9).

All tensors are bf16 on the wire and in SBUF (PSUM accumulates f32).
The per-head-pair score matmuls sit in disjoint PE row groups
(tile_position (0,0)/(64,0), K=64) so the hardware runs each hh pair
concurrently. Diagonal-straddle blocks stream only unmasked column
ranges (matmul start=True lazily zeroes the whole 2KB PSUM bank region,
so the mask matmul covers just the 128-col triangle straddle); fully
masked blocks are skipped.

The attention phase is software-pipelined as one flat stream: scores(i)
and exp(i) are emitted each step, and the attnV of step i-2 pops from a
pending queue, so the in-order PE never waits on the Act engine's exp.
V-projection k-tiles, per-pass normalizes (on an SBUF drain of the
attnV accumulator), and output-projection m-tiles interleave as PE
filler. PSUM: a single [128,1024]x2 ring (scores/V-proj/outproj/
broadcast) + [65,1024]x2 attnV accumulators = 8 banks exactly.
"""

import numpy as np

D_MODEL = 1024
N_HEADS = 16
D_K = 64
B, S = 2, 2048
N_CORES = 8
HPC = 4            # heads per core
KT = S // 128      # 16 k-tiles
QT = S // 512      # 4 q-tiles
ET = D_MODEL // 128  # 8 e-tiles (contraction tiles for projections)

_PROG_CACHE = {}


def _build_program():
    import concourse.bacc as bacc_mod
    import concourse.mybir as mybir
    import concourse.tile as tile

    f32 = mybir.dt.float32
    f32r = mybir.dt.float32r
    bf16 = mybir.dt.bfloat16
    Exp = mybir.ActivationFunctionType.Exp

    nc = bacc_mod.Bacc(
        "TRN2", target_bir_lowering=False, debug=False, num_devices=N_CORES
    )

    xq = nc.dram_tensor("xq", [D_MODEL, S], bf16, kind="ExternalInput").ap()
    xk = nc.dram_tensor("xk", [D_MODEL, S], bf16, kind="ExternalInput").ap()
    xv = nc.dram_tensor("xv", [D_MODEL, S], bf16, kind="ExternalInput").ap()
    wq = nc.dram_tensor("wq", [D_MODEL, 256], bf16, kind="ExternalInput").ap()
    wk = nc.dram_tensor("wk", [D_MODEL, 256], bf16, kind="ExternalInput").ap()
    wv = nc.dram_tensor("wv", [D_MODEL, 256], bf16, kind="ExternalInput").ap()
    wo = nc.dram_tensor("wo", [256, D_MODEL], bf16, kind="ExternalInput").ap()
    maskt = nc.dram_tensor("maskt", [128, 2048], bf16, kind="ExternalInput").ap()
    idbf = nc.dram_tensor("idbf", [128, 132], bf16, kind="ExternalInput").ap()
    consts = nc.dram_tensor("consts", [128, 193], f32r, kind="ExternalInput").ap()
    y = nc.dram_tensor("y", [S, D_MODEL], bf16, kind="ExternalOutput").ap()

    with (
        tile.TileContext(nc) as tc,
        nc.allow_low_precision("bf16 attention"),
        tc.tile_pool(name="persist", bufs=1) as pp,
    ):
        # ---- persistent SBUF tiles ----
        def persist(shape, dtype, name):
            return pp.tile(shape, dtype, name=name, tag=name)

        wq_sb = persist([128, ET * 256], bf16, "wq_sb")
        wk_sb = persist([128, ET * 256], bf16, "wk_sb")
        wv_sb = persist([128, ET * 256], bf16, "wv_sb")
        wo_sb = [persist([128, D_MODEL], bf16, f"wo_sb{p}") for p in range(2)]
        maskt_sb = persist([128, 2048], bf16, "maskt_sb")
        idbf_sb = persist([128, 132], bf16, "idbf_sb")
        consts_sb = persist([128, 193], f32r, "consts_sb")
        qt_sb = [persist([128, S], bf16, f"qt_sb{p}") for p in range(2)]
        kt_sb = [persist([128, S], bf16, f"kt_sb{p}") for p in range(2)]
        v_sb = [persist([128, 260], bf16, f"v_sb{i}") for i in range(KT)]
        outt_sb = [persist([128, S], bf16, f"outt_sb{p}") for p in range(2)]

        # DMA order: everything the attention stream's first steps need
        # comes first; consts/wo (first used tens of us in) ride last.
        nc.sync.dma_start(out=idbf_sb[:], in_=idbf[:])
        nc.sync.dma_start(
            out=wq_sb[:].rearrange("p (t d) -> p t d", t=ET),
            in_=wq.rearrange("(t p) d -> p t d", p=128),
        )

        # ---- PE warm-up ----
        # The PE HAM clock gate drops to K=4 half-clock after any multi-us
        # PE idle and needs ~3.4us of gapless activity to return to K=8.
        # idbf-only dummy matmuls (results never read) cover the initial
        # DMA ramp until the first projection e-tile lands.
        with tc.tile_pool(name="psW", bufs=1, space="PSUM") as psW:
            wt = psW.tile([128, 512], f32, name="warm", tag="warm")
            for w in range(28):
                nc.tensor.matmul(
                    wt[:, 0:132], idbf_sb[:, 0:128], idbf_sb[:, 0:132],
                    start=True, stop=True,
                )

        # ---- phase B: Q/K projections ----
        # Q^T/K^T accumulate over all 8 e-tiles into [128, 2048] PSUM (8
        # banks, both m-tiles), e-paced by the xq/xk DMA stream. The
        # PSUM->SBUF copies are split Act/DVE so the next tensor's e=0
        # matmuls (which reuse the banks) wait at most ~2us.
        xvkp_ctx = tc.tile_pool(name="xvk", bufs=4)
        xvkp = xvkp_ctx.__enter__()
        vdma_tiles = []

        def emit_v_dma():
            bidx = len(vdma_tiles)
            xvk = xvkp.tile([128, ET, 512], bf16, name=f"xvk_{bidx}", tag="xvk")
            eng = (nc.sync, nc.scalar, nc.gpsimd, nc.sync)[bidx]
            eng.dma_start(
                out=xvk[:],
                in_=xv[:, bidx * 512 : (bidx + 1) * 512].rearrange(
                    "(t p) k -> p t k", p=128
                ),
            )
            vdma_tiles.append(xvk)

        with tc.tile_pool(name="xe", bufs=3) as xep:
            psA_ctx = tc.tile_pool(name="psA", bufs=1, space="PSUM")
            psA = psA_ctx.__enter__()
            for ti, (x_dram, w_tile, dst) in enumerate(
                ((xq, wq_sb, qt_sb), (xk, wk_sb, kt_sb))
            ):
                ps = [
                    psA.tile(
                        [128, S], f32, name=f"ps_p{ti}_{m}", tag=f"proj{m}", bufs=1
                    )
                    for m in range(2)
                ]
                for e in range(ET):
                    xe = xep.tile([128, S], bf16, name=f"xe_{ti}_{e}", tag="xe")
                    eng = (nc.sync, nc.gpsimd, nc.scalar)[(ti * ET + e) % 3]
                    eng.dma_start(out=xe[:], in_=x_dram[e * 128 : (e + 1) * 128, :])
                    if ti == 0 and e == 1:
                        nc.scalar.dma_start(
                            out=wk_sb[:].rearrange("p (t d) -> p t d", t=ET),
                            in_=wk.rearrange("(t p) d -> p t d", p=128),
                        )
                    if ti == 0 and e == 3:
                        nc.scalar.dma_start(
                            out=wv_sb[:].rearrange("p (t d) -> p t d", t=ET),
                            in_=wv.rearrange("(t p) d -> p t d", p=128),
                        )
                    for _ in range(5):
                        nc.tensor.ldweights(idbf_sb[:, 0:128])
                    for m in range(2):
                        lhsT = w_tile[:, e * 256 + m * 128 : e * 256 + (m + 1) * 128]
                        for n in range(QT):
                            nc.tensor.matmul(
                                ps[m][:, n * 512 : (n + 1) * 512],
                                lhsT,
                                xe[:, n * 512 : (n + 1) * 512],
                                start=(e == 0),
                                stop=(e == ET - 1),
                            )
                nc.scalar.copy(dst[0][:], ps[0][:])
                nc.vector.tensor_copy(dst[1][:], ps[1][:])

            # post-projection loads, in first-use order
            nc.gpsimd.dma_start(out=maskt_sb[:], in_=maskt[:])
            emit_v_dma()  # xv batch 0
            emit_v_dma()  # xv batch 1
            emit_v_dma()  # xv batch 2
            emit_v_dma()  # xv batch 3
            nc.gpsimd.dma_start(out=consts_sb[:], in_=consts[:])
            for p in range(2):
                nc.gpsimd.dma_start(out=wo_sb[p][:], in_=wo[p * 128 : (p + 1) * 128, :])
            psA_ctx.__exit__(None, None, None)

        # ---- merged phase: V-projection + attention + outproj ----
        # PSUM: psS [128,1024]x2 (scores only, 4 banks) + psO [65,1024]x1
        # (attnV accumulator, 2 banks, drained to SBUF at pass end) +
        # psV [128,256]x2 (V-proj, 1 bank) which hands its bank to psY
        # [128,512]x2 (outproj halves + broadcast) once V-proj finishes.
        psS_ctx = tc.tile_pool(name="psS", bufs=2, space="PSUM")
        psS = psS_ctx.__enter__()
        psO_ctx = tc.tile_pool(name="psO", bufs=1, space="PSUM")
        psO = psO_ctx.__enter__()
        psV_ctx = tc.tile_pool(name="psV", bufs=2, space="PSUM")
        psV = psV_ctx.__enter__()
        psY = None  # opens once psV closes

        etp_ctx = tc.tile_pool(name="et", bufs=6)
        etp = etp_ctx.__enter__()
        obp_ctx = tc.tile_pool(name="ob", bufs=3)
        obp = obp_ctx.__enter__()
        bcp_ctx = tc.tile_pool(name="bcsb", bufs=2)
        bcp = bcp_ctx.__enter__()
        rcp_ctx = tc.tile_pool(name="rcsb", bufs=3)
        rcp = rcp_ctx.__enter__()
        ysb_ctx = tc.tile_pool(name="ysb", bufs=3)
        ysbp = ysb_ctx.__enter__()

        nvp = 0  # V-projection k-tiles emitted

        def emit_vproj_tile():
            nonlocal nvp
            i = nvp
            psv = psV.tile([128, 256], f32, name=f"psv_{i}", tag="v")
            xvk = vdma_tiles[i // 4]
            ks = slice((i % 4) * 128, (i % 4) * 128 + 128)
            for e in range(ET):
                nc.tensor.matmul(
                    psv[:],
                    xvk[:, e, ks],
                    wv_sb[:, e * 256 : (e + 1) * 256],
                    start=(e == 0),
                    stop=(e == ET - 1),
                )
            nc.vector.tensor_copy(
                v_sb[i][:].rearrange("p (h c) -> p h c", c=65)[:, :, 0:64],
                psv[:].rearrange("p (h d) -> p h d", d=64),
            )
            nc.vector.tensor_copy(
                v_sb[i][:].rearrange("p (h c) -> p h c", c=65)[:, :, 64:65],
                idbf_sb[:, 128:132].rearrange("p (h c) -> p h c", c=1),
            )
            nvp += 1

        def emit_warm_filler(count):
            # dependency-free dummy matmuls: keep the PE clock up across
            # unavoidable serial waits (final normalize chain)
            wt2 = psS.tile([128, 1024], f32, name="warm2", tag="s")
            for _ in range(count):
                nc.tensor.matmul(
                    wt2[:, 0:512], idbf_sb[:, 0:128], maskt_sb[:, 0:512],
                    start=True, stop=True,
                )

        def emit_normalize(pr, jj, ob):
            ssb = rcp.tile([33, 512], f32, name=f"ssb_{pr}_{jj}", tag="ssb")
            for hh in range(2):
                nc.vector.tensor_copy(
                    ssb[32 * hh : 32 * hh + 1, :],
                    ob[64:65, 512 * hh : 512 * (hh + 1)],
                )
            rc32 = rcp.tile([33, 512], f32, name=f"rc32_{pr}_{jj}", tag="rc32")
            nc.vector.reciprocal_approx_fast(out=rc32[:], in_=ssb[:])
            rc = rcp.tile([33, 512], f32r, name=f"rc_{pr}_{jj}", tag="rc")
            nc.vector.tensor_copy(rc[:], rc32[:])
            # broadcast rc across 64 partitions; the two heads' broadcast
            # matmuls sit in disjoint PE row groups (concurrent)
            bc_sb = bcp.tile([64, 1024], f32, name=f"bc_sb_{pr}_{jj}", tag="bc")
            for hh in range(2):
                bc = psY.tile([128, 512], f32, name=f"ps_bc_{pr}_{jj}_{hh}", tag="y")
                nc.tensor.matmul(
                    bc[0:64, :],
                    consts_sb[32 * hh : 32 * hh + 1, 128:192],
                    rc[32 * hh : 32 * hh + 1, :],
                    start=True,
                    stop=True,
                    tile_position=(32 * hh, 0),
                )
                nc.vector.tensor_copy(
                    bc_sb[:, 512 * hh : 512 * (hh + 1)],
                    bc[0:64, :],
                )
            # normalize per 128-col chunk so outproj m-tiles can chase
            for mo in range(4):
                for hh in range(2):
                    nc.vector.tensor_mul(
                        outt_sb[pr][64 * hh : 64 * hh + 64,
                                    jj * 512 + 128 * mo : jj * 512 + 128 * mo + 128],
                        ob[0:64, 512 * hh + 128 * mo : 512 * hh + 128 * mo + 128],
                        bc_sb[:, 512 * hh + 128 * mo : 512 * hh + 128 * mo + 128],
                    )

        ysb_tiles = {}

        def emit_outproj_half(m, n):
            psy = psY.tile([128, 512], f32, name=f"psy_{m}_{n}", tag="y")
            for p in range(2):
                nc.tensor.matmul(
                    psy[:],
                    outt_sb[p][:, m * 128 : (m + 1) * 128],
                    wo_sb[p][:, n * 512 : (n + 1) * 512],
                    start=(p == 0),
                    stop=(p == 1),
                )
            if n == 0:
                ysb_tiles[m] = ysbp.tile(
                    [128, 1024], bf16, name=f"y_sb_{m}", tag="ysb"
                )
            y_sb = ysb_tiles[m]
            nc.vector.tensor_copy(y_sb[:, n * 512 : (n + 1) * 512], psy[:])
            if n == 1:
                nc.gpsimd.dma_start(out=y[m * 128 : (m + 1) * 128, :], in_=y_sb[:])

        # ---- the flat attention stream ----
        # pr0 leads with j=1 (its first 4 k-tiles are mask-free, buying
        # the maskt DMA time); pr1 ascends so outproj work spreads through
        # the stream instead of piling past the last exp.
        passes = [(0, 1), (0, 0), (0, 2), (0, 3)] + [(1, j) for j in range(QT)]
        steps = []
        for pr, j in passes:
            for i in range(4 * j + 4):
                steps.append((pr, j, i))

        # earliest step at which V-proj k-tile t may be emitted (its xv
        # batch t//4 needs to have landed; consumption is later still)
        vp_sched = {0: 1, 1: 2, 2: 3, 3: 4, 4: 5, 5: 6, 6: 7, 7: 8,
                    8: 14, 9: 16, 10: 18, 11: 20,
                    12: 28, 13: 30, 14: 32, 15: 34}

        boundary_fill = [0]  # steps remaining of post-drain LDW filler
        ps_outs = {}   # (pr, j) -> psO tile
        ob_tiles = {}  # (pr, j) -> SBUF drain tile
        pending = []   # emitted exps awaiting their attnV
        norm_q = []    # (ready_step, pr, j)
        outp_q = []    # (ready_step, m, n) output-projection halves
        pop_hold = 0   # extra pop delay right after a drain (psO WAR)

        def emit_attnv(pr, j, i, et, c0, n_i):
            nonlocal pop_hold
            ps_out = ps_outs[(pr, j)]
            for hh in range(2):
                nc.tensor.matmul(
                    ps_out[:, 512 * hh + c0 : 512 * (hh + 1)],
                    v_sb[i][:, (2 * pr + hh) * 65 : (2 * pr + hh + 1) * 65],
                    et[:, 512 * hh + c0 : 512 * (hh + 1)],
                    start=(i == 0),
                    stop=(i == n_i - 1),
                    skip_group_check=True,
                )
            if i == n_i - 1:
                # pass complete: drain the accumulator to SBUF and queue
                # the (fully deferrable) normalize
                ob = obp.tile([65, 1024], f32, name=f"ob_{pr}_{j}", tag="ob")
                nc.vector.tensor_copy(ob[:], ps_out[:])
                ob_tiles[(pr, j)] = ob
                pop_hold = 1
                boundary_fill[0] = 3

        emit_warm_filler(10)

        for sidx, (pr, j, i) in enumerate(steps):
            n_i = 4 * j + 4
            if i == 0:
                ps_outs[(pr, j)] = psO.tile(
                    [65, 1024], f32, name=f"ps_out_{pr}_{j}", tag="o"
                )
            if boundary_fill[0] > 0:
                boundary_fill[0] -= 1
                for _ in range(4):
                    nc.tensor.ldweights(idbf_sb[:, 0:128])

            # scores (+ causal mask straddle) and exp
            diag = i >= 4 * j
            r = i - 4 * j
            c0 = 128 * r if diag else 0
            qs = slice(j * 512, (j + 1) * 512)
            pss = psS.tile([128, 1024], f32, name=f"ps_s{pr}_{j}_{i}", tag="s")
            if diag:
                for hh in range(2):
                    nc.tensor.matmul(
                        pss[:, 512 * hh + c0 : 512 * hh + c0 + 128],
                        idbf_sb[:, 0:128],
                        maskt_sb[:, r * 512 + c0 : r * 512 + c0 + 128],
                        start=True,
                        stop=False,
                    )
            for hh in range(2):
                hp = slice(64 * hh, 64 * hh + 64)
                nc.tensor.matmul(
                    pss[:, 512 * hh + c0 : 512 * (hh + 1)],
                    kt_sb[pr][hp, i * 128 : (i + 1) * 128],
                    qt_sb[pr][hp, qs.start + c0 : qs.stop],
                    start=not diag,
                    stop=True,
                    skip_group_check=diag,
                )
            et = etp.tile([128, 1024], bf16, name=f"et{pr}_{j}_{i}", tag="et")
            nc.scalar.activation(et[:, c0:1024], pss[:, c0:1024], Exp, scale=0.125)
            pending.append((pr, j, i, et, c0, n_i))
            if len(pending) >= 3 + pop_hold:
                emit_attnv(*pending.pop(0))
            elif pop_hold:
                pop_hold = 0
            if i == n_i - 1:
                norm_q.append((sidx + 3, pr, j))

            # PE filler after this step's main work
            if (
                norm_q
                and norm_q[0][0] <= sidx
                and psY is not None
                and tuple(norm_q[0][1:]) in ob_tiles
            ):
                _, npr, nj = norm_q.pop(0)
                emit_normalize(npr, nj, ob_tiles.pop((npr, nj)))
                if npr == 1:
                    for mo in range(4):
                        for n in range(2):
                            outp_q.append((sidx + 2 + mo, 4 * nj + mo, n))
            if nvp < KT and vp_sched[nvp] <= sidx:
                emit_vproj_tile()
            elif outp_q and outp_q[0][0] <= sidx:
                _, m, n = outp_q.pop(0)
                emit_outproj_half(m, n)
                # one more half if backlogged
                if outp_q and outp_q[0][0] + 2 <= sidx:
                    _, m, n = outp_q.pop(0)
                    emit_outproj_half(m, n)
            if nvp == KT and psY is None:
                psV_ctx.__exit__(None, None, None)
                psY_ctx = tc.tile_pool(name="psY", bufs=2, space="PSUM")
                psY = psY_ctx.__enter__()

        # tail: flush remaining attnVs, final normalize + outproj chase,
        # with warm filler keeping the PE clock up through the DVE chain
        while pending:
            emit_attnv(*pending.pop(0))
        emit_warm_filler(6)
        while norm_q:
            _, npr, nj = norm_q.pop(0)
            emit_normalize(npr, nj, ob_tiles.pop((npr, nj)))
            if npr == 1:
                for mo in range(4):
                    for n in range(2):
                        outp_q.append((0, 4 * nj + mo, n))
            emit_warm_filler(4)
        while outp_q:
            _, m, n = outp_q.pop(0)
            emit_outproj_half(m, n)

        for ctx in (ysb_ctx, rcp_ctx, bcp_ctx, obp_ctx, etp_ctx):
            ctx.__exit__(None, None, None)
        psY_ctx.__exit__(None, None, None)
        psO_ctx.__exit__(None, None, None)
        psS_ctx.__exit__(None, None, None)
        xvkp_ctx.__exit__(None, None, None)

    nc.compile()
    return nc


def _get_program():
    if "nc" not in _PROG_CACHE:
        _PROG_CACHE["nc"] = _build_program()
    return _PROG_CACHE["nc"]


def _host_prep(query, key, value, mask, w_q, w_k, w_v, w_o):
    import ml_dtypes

    bf = ml_dtypes.bfloat16
    query = np.asarray(query, dtype=np.float32)
    key = np.asarray(key, dtype=np.float32)
    value = np.asarray(value, dtype=np.float32)
    w_q = np.asarray(w_q, dtype=np.float32)
    w_k = np.asarray(w_k, dtype=np.float32)
    w_v = np.asarray(w_v, dtype=np.float32)
    w_o = np.asarray(w_o, dtype=np.float32)
    m = np.asarray(mask).reshape(S, S).astype(bool)

    # The kernel's block-skip structure assumes the standard causal mask.
    expected = np.triu(np.ones((S, S), dtype=bool), k=1)
    if not np.array_equal(m, expected):
        raise NotImplementedError("kernel specialized for causal (triu, k=1) mask")

    # 4 canonical diagonal-straddle mask tiles: pattern r covers k-tile
    # 4j+r vs q-tile j; masked where (128r + row) > col.
    maskt = np.zeros((128, 2048), dtype=np.float32)
    rows = np.arange(128)[:, None]
    cols = np.arange(512)[None, :]
    for r in range(4):
        maskt[:, r * 512 : (r + 1) * 512] = np.where(
            (128 * r + rows) > cols, np.float32(-1e9), np.float32(0.0)
        )
    maskt = maskt.astype(bf)
    idbf = np.zeros((128, 132), dtype=bf)
    idbf[:, 0:128] = np.eye(128, dtype=bf)
    idbf[:, 128:132] = bf(1.0)

    consts = np.zeros((128, 193), dtype=np.float32)
    consts[:, 0:128] = np.eye(128, dtype=np.float32)
    consts[:, 128:193] = 1.0

    xt = {}
    for b in range(B):
        xt[("q", b)] = np.ascontiguousarray(query[b].T.astype(bf))
        xt[("k", b)] = np.ascontiguousarray(key[b].T.astype(bf))
        xt[("v", b)] = np.ascontiguousarray(value[b].T.astype(bf))

    in_maps = []
    for c in range(N_CORES):
        b = c // 4
        hb = (c % 4) * HPC
        rs = slice(hb * D_K, (hb + HPC) * D_K)
        in_maps.append(
            {
                "xq": xt[("q", b)],
                "xk": xt[("k", b)],
                "xv": xt[("v", b)],
                "wq": np.ascontiguousarray(w_q[rs, :].T.astype(bf)),
                "wk": np.ascontiguousarray(w_k[rs, :].T.astype(bf)),
                "wv": np.ascontiguousarray(w_v[rs, :].T.astype(bf)),
                "wo": np.ascontiguousarray(w_o[:, rs].T.astype(bf)),
                "maskt": maskt,
                "idbf": idbf,
                "consts": consts,
            }
        )
    return in_maps


def kernel(query, key, value, mask, w_q, w_k, w_v, w_o):
    from concourse.bass_utils import run_bass_kernel_spmd

    in_maps = _host_prep(query, key, value, mask, w_q, w_k, w_v, w_o)
    nc = _get_program()
    res = run_bass_kernel_spmd(nc, in_maps, list(range(N_CORES)))
    out = np.zeros((B, S, D_MODEL), dtype=np.float32)
    for c in range(N_CORES):
        out[c // 4] += np.asarray(res.results[c]["y"], dtype=np.float32)
    return out


# revision 20
# speedup vs baseline: 1.0510x; 1.0510x over previous
"""Multi-head causal attention (B=2, S=2048, D=1024, H=16) on 8 TRN2 NeuronCores.

Sharding: batch*head parallel. Core c handles batch b = c//4 and the 4
heads h in [4*(c%4), 4*(c%4)+4). Each core computes its heads' Q/K/V
projections (column-parallel), causal softmax attention, and its partial
row-parallel output projection; the host sums the 4 partial outputs per
batch (the AllReduce of row-parallel tensor parallelism).

On-device layout: everything is kept "transposed" (feature-major) so
every matmul contracts along the partition dimension:
  scoresT[k,q] = K Q^T      (per head, 128-row k-tiles x 512-col q-tiles)
  P^T = exp(scoresT/8 + mask/8)   (additive -1e9 causal mask)
  outT[d,q]   = sum_k V[k,d] P^T[k,q]   (PSUM-accumulated over k-tiles)
  sums[q]     = sum_k P^T[k,q]          (ones-vector matmul, col-packed)
  y[q,e]     += sum_hd outT_norm[hd,q] * w_oT[hd,e]
Softmax skips the max-subtraction: scores ~ N(0,1) so exp never
overflows, and exp(-1e9/8) underflows to exactly 0 like the
reference's masked_fill(-1e9).

All tensors are bf16 on the wire and in SBUF (PSUM accumulates f32).
The per-head-pair score matmuls sit in disjoint PE row groups
(tile_position (0,0)/(64,0), K=64) so the hardware runs each hh pair
concurrently. Diagonal-straddle blocks stream only unmasked column
ranges (matmul start=True lazily zeroes the whole 2KB PSUM bank region,
so the mask matmul covers just the 128-col triangle straddle); fully
masked blocks are skipped.

The attention phase is software-pipelined as one flat stream: scores(i)
and exp(i) are emitted each step, and the attnV of step i-2 pops from a
pending queue, so the in-order PE never waits on the Act engine's exp.
V-projection k-tiles, per-pass normalizes (on an SBUF drain of the
attnV accumulator), and output-projection m-tiles interleave as PE
filler. PSUM: a single [128,1024]x2 ring (scores/V-proj/outproj/
broadcast) + [65,1024]x2 attnV accumulators = 8 banks exactly.
"""

import numpy as np

D_MODEL = 1024
N_HEADS = 16
D_K = 64
B, S = 2, 2048
N_CORES = 8
HPC = 4            # heads per core
KT = S // 128      # 16 k-tiles
QT = S // 512      # 4 q-tiles
ET = D_MODEL // 128  # 8 e-tiles (contraction tiles for projections)

_PROG_CACHE = {}


def _build_program():
    import concourse.bacc as bacc_mod
    import concourse.mybir as mybir
    import concourse.tile as tile

    f32 = mybir.dt.float32
    f32r = mybir.dt.float32r
    bf16 = mybir.dt.bfloat16
    Exp = mybir.ActivationFunctionType.Exp

    nc = bacc_mod.Bacc(
        "TRN2", target_bir_lowering=False, debug=False, num_devices=N_CORES
    )

    xq = nc.dram_tensor("xq", [D_MODEL, S], bf16, kind="ExternalInput").ap()
    xk = nc.dram_tensor("xk", [D_MODEL, S], bf16, kind="ExternalInput").ap()
    xv = nc.dram_tensor("xv", [D_MODEL, S], bf16, kind="ExternalInput").ap()
    wq = nc.dram_tensor("wq", [D_MODEL, 256], bf16, kind="ExternalInput").ap()
    wk = nc.dram_tensor("wk", [D_MODEL, 256], bf16, kind="ExternalInput").ap()
    wv = nc.dram_tensor("wv", [D_MODEL, 256], bf16, kind="ExternalInput").ap()
    wo = nc.dram_tensor("wo", [256, D_MODEL], bf16, kind="ExternalInput").ap()
    maskt = nc.dram_tensor("maskt", [128, 2048], bf16, kind="ExternalInput").ap()
    idbf = nc.dram_tensor("idbf", [128, 132], bf16, kind="ExternalInput").ap()
    consts = nc.dram_tensor("consts", [128, 193], f32r, kind="ExternalInput").ap()
    y = nc.dram_tensor("y", [S, D_MODEL], bf16, kind="ExternalOutput").ap()

    with (
        tile.TileContext(nc) as tc,
        nc.allow_low_precision("bf16 attention"),
        tc.tile_pool(name="persist", bufs=1) as pp,
    ):
        # ---- persistent SBUF tiles ----
        def persist(shape, dtype, name):
            return pp.tile(shape, dtype, name=name, tag=name)

        wq_sb = persist([128, ET * 256], bf16, "wq_sb")
        wk_sb = persist([128, ET * 256], bf16, "wk_sb")
        wv_sb = persist([128, ET * 256], bf16, "wv_sb")
        wo_sb = [persist([128, D_MODEL], bf16, f"wo_sb{p}") for p in range(2)]
        maskt_sb = persist([128, 2048], bf16, "maskt_sb")
        idbf_sb = persist([128, 132], bf16, "idbf_sb")
        consts_sb = persist([128, 193], f32r, "consts_sb")
        qt_sb = [persist([128, S], bf16, f"qt_sb{p}") for p in range(2)]
        kt_sb = [persist([128, S], bf16, f"kt_sb{p}") for p in range(2)]
        v_sb = [persist([128, 260], bf16, f"v_sb{i}") for i in range(KT)]
        outt_sb = [persist([128, S], bf16, f"outt_sb{p}") for p in range(2)]

        # DMA order: everything the attention stream's first steps need
        # comes first; consts/wo (first used tens of us in) ride last.
        nc.sync.dma_start(out=idbf_sb[:], in_=idbf[:])
        nc.sync.dma_start(
            out=wq_sb[:].rearrange("p (t d) -> p t d", t=ET),
            in_=wq.rearrange("(t p) d -> p t d", p=128),
        )

        # ---- PE warm-up ----
        # The PE HAM clock gate drops to K=4 half-clock after any multi-us
        # PE idle and needs ~3.4us of gapless activity to return to K=8.
        # idbf-only dummy matmuls (results never read) cover the initial
        # DMA ramp until the first projection e-tile lands.
        with tc.tile_pool(name="psW", bufs=1, space="PSUM") as psW:
            wt = psW.tile([128, 512], f32, name="warm", tag="warm")
            for w in range(28):
                nc.tensor.matmul(
                    wt[:, 0:132], idbf_sb[:, 0:128], idbf_sb[:, 0:132],
                    start=True, stop=True,
                )

        # ---- phase B: Q/K projections ----
        # Q^T/K^T accumulate over all 8 e-tiles into [128, 2048] PSUM (8
        # banks, both m-tiles), e-paced by the xq/xk DMA stream. The
        # PSUM->SBUF copies are split Act/DVE so the next tensor's e=0
        # matmuls (which reuse the banks) wait at most ~2us.
        xvkp_ctx = tc.tile_pool(name="xvk", bufs=4)
        xvkp = xvkp_ctx.__enter__()
        vdma_tiles = []

        def emit_v_dma():
            bidx = len(vdma_tiles)
            xvk = xvkp.tile([128, ET, 512], bf16, name=f"xvk_{bidx}", tag="xvk")
            eng = (nc.sync, nc.scalar, nc.sync, nc.scalar)[bidx]
            eng.dma_start(
                out=xvk[:],
                in_=xv[:, bidx * 512 : (bidx + 1) * 512].rearrange(
                    "(t p) k -> p t k", p=128
                ),
            )
            vdma_tiles.append(xvk)

        with tc.tile_pool(name="xe", bufs=6) as xep:
            # Issue EVERY phase-B load up front, round-robin across the
            # three DMA queues (SP / Pool / Act), highest-priority first
            # per queue. No issue ever sits behind a compute op in a
            # sequencer stream, and the three hardware queues stream in
            # parallel the whole phase.
            xe_tiles = {}
            engs = (nc.sync, nc.gpsimd, nc.scalar)
            for ti, x_dram in enumerate((xq, xk)):
                for e in range(ET):
                    xe_tiles[(ti, e)] = xep.tile(
                        [128, S], bf16, name=f"xe_{ti}_{e}", tag="xe"
                    )
            for e in range(ET):
                engs[e % 3].dma_start(
                    out=xe_tiles[(0, e)][:], in_=xq[e * 128 : (e + 1) * 128, :]
                )
            nc.scalar.dma_start(
                out=wk_sb[:].rearrange("p (t d) -> p t d", t=ET),
                in_=wk.rearrange("(t p) d -> p t d", p=128),
            )
            for e in range(ET):
                engs[(ET + e) % 3].dma_start(
                    out=xe_tiles[(1, e)][:], in_=xk[e * 128 : (e + 1) * 128, :]
                )
            nc.scalar.dma_start(
                out=wv_sb[:].rearrange("p (t d) -> p t d", t=ET),
                in_=wv.rearrange("(t p) d -> p t d", p=128),
            )
            nc.gpsimd.dma_start(out=maskt_sb[:], in_=maskt[:])
            emit_v_dma()  # xv batches 0-3
            emit_v_dma()
            emit_v_dma()
            emit_v_dma()
            nc.gpsimd.dma_start(out=consts_sb[:], in_=consts[:])
            for p in range(2):
                nc.gpsimd.dma_start(out=wo_sb[p][:], in_=wo[p * 128 : (p + 1) * 128, :])

            psA_ctx = tc.tile_pool(name="psA", bufs=1, space="PSUM")
            psA = psA_ctx.__enter__()
            for ti, (w_tile, dst) in enumerate(
                ((wq_sb, qt_sb), (wk_sb, kt_sb))
            ):
                ps = [
                    psA.tile(
                        [128, S], f32, name=f"ps_p{ti}_{m}", tag=f"proj{m}", bufs=1
                    )
                    for m in range(2)
                ]
                for e in range(ET):
                    xe = xe_tiles[(ti, e)]
                    for _ in range(5):
                        nc.tensor.ldweights(idbf_sb[:, 0:128])
                    for m in range(2):
                        lhsT = w_tile[:, e * 256 + m * 128 : e * 256 + (m + 1) * 128]
                        for n in range(QT):
                            nc.tensor.matmul(
                                ps[m][:, n * 512 : (n + 1) * 512],
                                lhsT,
                                xe[:, n * 512 : (n + 1) * 512],
                                start=(e == 0),
                                stop=(e == ET - 1),
                            )
                nc.scalar.copy(dst[0][:], ps[0][:])
                nc.vector.tensor_copy(dst[1][:], ps[1][:])
            psA_ctx.__exit__(None, None, None)

        # ---- merged phase: V-projection + attention + outproj ----
        # PSUM: psS [128,1024]x2 (scores only, 4 banks) + psO [65,1024]x1
        # (attnV accumulator, 2 banks, drained to SBUF at pass end) +
        # psV [128,256]x2 (V-proj, 1 bank) which hands its bank to psY
        # [128,512]x2 (outproj halves + broadcast) once V-proj finishes.
        psS_ctx = tc.tile_pool(name="psS", bufs=2, space="PSUM")
        psS = psS_ctx.__enter__()
        psO_ctx = tc.tile_pool(name="psO", bufs=1, space="PSUM")
        psO = psO_ctx.__enter__()
        psV_ctx = tc.tile_pool(name="psV", bufs=2, space="PSUM")
        psV = psV_ctx.__enter__()
        psY = None  # opens once psV closes

        etp_ctx = tc.tile_pool(name="et", bufs=6)
        etp = etp_ctx.__enter__()
        obp_ctx = tc.tile_pool(name="ob", bufs=3)
        obp = obp_ctx.__enter__()
        bcp_ctx = tc.tile_pool(name="bcsb", bufs=2)
        bcp = bcp_ctx.__enter__()
        rcp_ctx = tc.tile_pool(name="rcsb", bufs=3)
        rcp = rcp_ctx.__enter__()
        ysb_ctx = tc.tile_pool(name="ysb", bufs=3)
        ysbp = ysb_ctx.__enter__()

        nvp = 0  # V-projection k-tiles emitted

        def emit_vproj_tile():
            nonlocal nvp
            i = nvp
            psv = psV.tile([128, 256], f32, name=f"psv_{i}", tag="v")
            xvk = vdma_tiles[i // 4]
            ks = slice((i % 4) * 128, (i % 4) * 128 + 128)
            for e in range(ET):
                nc.tensor.matmul(
                    psv[:],
                    xvk[:, e, ks],
                    wv_sb[:, e * 256 : (e + 1) * 256],
                    start=(e == 0),
                    stop=(e == ET - 1),
                )
            nc.vector.tensor_copy(
                v_sb[i][:].rearrange("p (h c) -> p h c", c=65)[:, :, 0:64],
                psv[:].rearrange("p (h d) -> p h d", d=64),
            )
            nc.vector.tensor_copy(
                v_sb[i][:].rearrange("p (h c) -> p h c", c=65)[:, :, 64:65],
                idbf_sb[:, 128:132].rearrange("p (h c) -> p h c", c=1),
            )
            nvp += 1

        def emit_warm_filler(count):
            # dependency-free dummy matmuls: keep the PE clock up across
            # unavoidable serial waits (final normalize chain)
            wt2 = psS.tile([128, 1024], f32, name="warm2", tag="s")
            for _ in range(count):
                nc.tensor.matmul(
                    wt2[:, 0:512], idbf_sb[:, 0:128], maskt_sb[:, 0:512],
                    start=True, stop=True,
                )

        def emit_normalize(pr, jj, ob):
            ssb = rcp.tile([33, 512], f32, name=f"ssb_{pr}_{jj}", tag="ssb")
            for hh in range(2):
                nc.vector.tensor_copy(
                    ssb[32 * hh : 32 * hh + 1, :],
                    ob[64:65, 512 * hh : 512 * (hh + 1)],
                )
            rc32 = rcp.tile([33, 512], f32, name=f"rc32_{pr}_{jj}", tag="rc32")
            nc.vector.reciprocal_approx_fast(out=rc32[:], in_=ssb[:])
            rc = rcp.tile([33, 512], f32r, name=f"rc_{pr}_{jj}", tag="rc")
            nc.vector.tensor_copy(rc[:], rc32[:])
            # broadcast rc across 64 partitions; the two heads' broadcast
            # matmuls sit in disjoint PE row groups (concurrent)
            bc_sb = bcp.tile([64, 1024], f32, name=f"bc_sb_{pr}_{jj}", tag="bc")
            for hh in range(2):
                bc = psY.tile([128, 512], f32, name=f"ps_bc_{pr}_{jj}_{hh}", tag="y")
                nc.tensor.matmul(
                    bc[0:64, :],
                    consts_sb[32 * hh : 32 * hh + 1, 128:192],
                    rc[32 * hh : 32 * hh + 1, :],
                    start=True,
                    stop=True,
                    tile_position=(32 * hh, 0),
                )
                nc.vector.tensor_copy(
                    bc_sb[:, 512 * hh : 512 * (hh + 1)],
                    bc[0:64, :],
                )
            # normalize per 128-col chunk so outproj m-tiles can chase
            for mo in range(4):
                for hh in range(2):
                    nc.vector.tensor_mul(
                        outt_sb[pr][64 * hh : 64 * hh + 64,
                                    jj * 512 + 128 * mo : jj * 512 + 128 * mo + 128],
                        ob[0:64, 512 * hh + 128 * mo : 512 * hh + 128 * mo + 128],
                        bc_sb[:, 512 * hh + 128 * mo : 512 * hh + 128 * mo + 128],
                    )

        ysb_tiles = {}

        def emit_outproj_half(m, n):
            psy = psY.tile([128, 512], f32, name=f"psy_{m}_{n}", tag="y")
            for p in range(2):
                nc.tensor.matmul(
                    psy[:],
                    outt_sb[p][:, m * 128 : (m + 1) * 128],
                    wo_sb[p][:, n * 512 : (n + 1) * 512],
                    start=(p == 0),
                    stop=(p == 1),
                )
            if n == 0:
                ysb_tiles[m] = ysbp.tile(
                    [128, 1024], bf16, name=f"y_sb_{m}", tag="ysb"
                )
            y_sb = ysb_tiles[m]
            nc.vector.tensor_copy(y_sb[:, n * 512 : (n + 1) * 512], psy[:])
            if n == 1:
                nc.gpsimd.dma_start(out=y[m * 128 : (m + 1) * 128, :], in_=y_sb[:])

        # ---- the flat attention stream ----
        # pr0 leads with j=1 (its first 4 k-tiles are mask-free, buying
        # the maskt DMA time); pr1 ascends so outproj work spreads through
        # the stream instead of piling past the last exp.
        passes = [(0, 1), (0, 0), (0, 2), (0, 3)] + [(1, j) for j in range(QT)]
        steps = []
        for pr, j in passes:
            for i in range(4 * j + 4):
                steps.append((pr, j, i))

        # earliest step at which V-proj k-tile t may be emitted (its xv
        # batch t//4 needs to have landed; consumption is later still)
        vp_sched = {0: 1, 1: 2, 2: 3, 3: 4, 4: 5, 5: 6, 6: 7, 7: 8,
                    8: 14, 9: 16, 10: 18, 11: 20,
                    12: 28, 13: 30, 14: 32, 15: 34}

        boundary_fill = [0]  # steps remaining of post-drain LDW filler
        ps_outs = {}   # (pr, j) -> psO tile
        ob_tiles = {}  # (pr, j) -> SBUF drain tile
        pending = []   # emitted exps awaiting their attnV
        norm_q = []    # (ready_step, pr, j)
        outp_q = []    # (ready_step, m, n) output-projection halves
        pop_hold = 0   # extra pop delay right after a drain (psO WAR)

        def emit_attnv(pr, j, i, et, c0, n_i):
            nonlocal pop_hold
            ps_out = ps_outs[(pr, j)]
            for hh in range(2):
                nc.tensor.matmul(
                    ps_out[:, 512 * hh + c0 : 512 * (hh + 1)],
                    v_sb[i][:, (2 * pr + hh) * 65 : (2 * pr + hh + 1) * 65],
                    et[:, 512 * hh + c0 : 512 * (hh + 1)],
                    start=(i == 0),
                    stop=(i == n_i - 1),
                    skip_group_check=True,
                )
            if i == n_i - 1:
                # pass complete: drain the accumulator to SBUF and queue
                # the (fully deferrable) normalize
                ob = obp.tile([65, 1024], f32, name=f"ob_{pr}_{j}", tag="ob")
                nc.vector.tensor_copy(ob[:], ps_out[:])
                ob_tiles[(pr, j)] = ob
                pop_hold = 1
                boundary_fill[0] = 3

        emit_warm_filler(10)

        for sidx, (pr, j, i) in enumerate(steps):
            n_i = 4 * j + 4
            if i == 0:
                ps_outs[(pr, j)] = psO.tile(
                    [65, 1024], f32, name=f"ps_out_{pr}_{j}", tag="o"
                )
            if boundary_fill[0] > 0:
                boundary_fill[0] -= 1
                for _ in range(4):
                    nc.tensor.ldweights(idbf_sb[:, 0:128])

            # scores (+ causal mask straddle) and exp
            diag = i >= 4 * j
            r = i - 4 * j
            c0 = 128 * r if diag else 0
            qs = slice(j * 512, (j + 1) * 512)
            pss = psS.tile([128, 1024], f32, name=f"ps_s{pr}_{j}_{i}", tag="s")
            if diag:
                for hh in range(2):
                    nc.tensor.matmul(
                        pss[:, 512 * hh + c0 : 512 * hh + c0 + 128],
                        idbf_sb[:, 0:128],
                        maskt_sb[:, r * 512 + c0 : r * 512 + c0 + 128],
                        start=True,
                        stop=False,
                    )
            for hh in range(2):
                hp = slice(64 * hh, 64 * hh + 64)
                nc.tensor.matmul(
                    pss[:, 512 * hh + c0 : 512 * (hh + 1)],
                    kt_sb[pr][hp, i * 128 : (i + 1) * 128],
                    qt_sb[pr][hp, qs.start + c0 : qs.stop],
                    start=not diag,
                    stop=True,
                    skip_group_check=diag,
                )
            et = etp.tile([128, 1024], bf16, name=f"et{pr}_{j}_{i}", tag="et")
            nc.scalar.activation(et[:, c0:1024], pss[:, c0:1024], Exp, scale=0.125)
            pending.append((pr, j, i, et, c0, n_i))
            if len(pending) >= 3 + pop_hold:
                emit_attnv(*pending.pop(0))
            elif pop_hold:
                pop_hold = 0
            if i == n_i - 1:
                norm_q.append((sidx + 3, pr, j))

            # PE filler after this step's main work
            if (
                norm_q
                and norm_q[0][0] <= sidx
                and psY is not None
                and tuple(norm_q[0][1:]) in ob_tiles
            ):
                _, npr, nj = norm_q.pop(0)
                emit_normalize(npr, nj, ob_tiles.pop((npr, nj)))
                if npr == 1:
                    for mo in range(4):
                        for n in range(2):
                            outp_q.append((sidx + 2 + mo, 4 * nj + mo, n))
            if nvp < KT and vp_sched[nvp] <= sidx:
                emit_vproj_tile()
            elif outp_q and outp_q[0][0] <= sidx:
                _, m, n = outp_q.pop(0)
                emit_outproj_half(m, n)
                # one more half if backlogged
                if outp_q and outp_q[0][0] + 2 <= sidx:
                    _, m, n = outp_q.pop(0)
                    emit_outproj_half(m, n)
            if nvp == KT and psY is None:
                psV_ctx.__exit__(None, None, None)
                psY_ctx = tc.tile_pool(name="psY", bufs=2, space="PSUM")
                psY = psY_ctx.__enter__()

        # tail: flush remaining attnVs, final normalize + outproj chase,
        # with warm filler keeping the PE clock up through the DVE chain
        while pending:
            emit_attnv(*pending.pop(0))
        emit_warm_filler(6)
        while norm_q:
            _, npr, nj = norm_q.pop(0)
            emit_normalize(npr, nj, ob_tiles.pop((npr, nj)))
            if npr == 1:
                for mo in range(4):
                    for n in range(2):
                        outp_q.append((0, 4 * nj + mo, n))
            emit_warm_filler(4)
        while outp_q:
            _, m, n = outp_q.pop(0)
            emit_outproj_half(m, n)

        for ctx in (ysb_ctx, rcp_ctx, bcp_ctx, obp_ctx, etp_ctx):
            ctx.__exit__(None, None, None)
        psY_ctx.__exit__(None, None, None)
        psO_ctx.__exit__(None, None, None)
        psS_ctx.__exit__(None, None, None)
        xvkp_ctx.__exit__(None, None, None)

    nc.compile()
    return nc


def _get_program():
    if "nc" not in _PROG_CACHE:
        _PROG_CACHE["nc"] = _build_program()
    return _PROG_CACHE["nc"]


def _host_prep(query, key, value, mask, w_q, w_k, w_v, w_o):
    import ml_dtypes

    bf = ml_dtypes.bfloat16
    query = np.asarray(query, dtype=np.float32)
    key = np.asarray(key, dtype=np.float32)
    value = np.asarray(value, dtype=np.float32)
    w_q = np.asarray(w_q, dtype=np.float32)
    w_k = np.asarray(w_k, dtype=np.float32)
    w_v = np.asarray(w_v, dtype=np.float32)
    w_o = np.asarray(w_o, dtype=np.float32)
    m = np.asarray(mask).reshape(S, S).astype(bool)

    # The kernel's block-skip structure assumes the standard causal mask.
    expected = np.triu(np.ones((S, S), dtype=bool), k=1)
    if not np.array_equal(m, expected):
        raise NotImplementedError("kernel specialized for causal (triu, k=1) mask")

    # 4 canonical diagonal-straddle mask tiles: pattern r covers k-tile
    # 4j+r vs q-tile j; masked where (128r + row) > col.
    maskt = np.zeros((128, 2048), dtype=np.float32)
    rows = np.arange(128)[:, None]
    cols = np.arange(512)[None, :]
    for r in range(4):
        maskt[:, r * 512 : (r + 1) * 512] = np.where(
            (128 * r + rows) > cols, np.float32(-1e9), np.float32(0.0)
        )
    maskt = maskt.astype(bf)
    idbf = np.zeros((128, 132), dtype=bf)
    idbf[:, 0:128] = np.eye(128, dtype=bf)
    idbf[:, 128:132] = bf(1.0)

    consts = np.zeros((128, 193), dtype=np.float32)
    consts[:, 0:128] = np.eye(128, dtype=np.float32)
    consts[:, 128:193] = 1.0

    xt = {}
    for b in range(B):
        xt[("q", b)] = np.ascontiguousarray(query[b].T.astype(bf))
        xt[("k", b)] = np.ascontiguousarray(key[b].T.astype(bf))
        xt[("v", b)] = np.ascontiguousarray(value[b].T.astype(bf))

    in_maps = []
    for c in range(N_CORES):
        b = c // 4
        hb = (c % 4) * HPC
        rs = slice(hb * D_K, (hb + HPC) * D_K)
        in_maps.append(
            {
                "xq": xt[("q", b)],
                "xk": xt[("k", b)],
                "xv": xt[("v", b)],
                "wq": np.ascontiguousarray(w_q[rs, :].T.astype(bf)),
                "wk": np.ascontiguousarray(w_k[rs, :].T.astype(bf)),
                "wv": np.ascontiguousarray(w_v[rs, :].T.astype(bf)),
                "wo": np.ascontiguousarray(w_o[:, rs].T.astype(bf)),
                "maskt": maskt,
                "idbf": idbf,
                "consts": consts,
            }
        )
    return in_maps


def kernel(query, key, value, mask, w_q, w_k, w_v, w_o):
    from concourse.bass_utils import run_bass_kernel_spmd

    in_maps = _host_prep(query, key, value, mask, w_q, w_k, w_v, w_o)
    nc = _get_program()
    res = run_bass_kernel_spmd(nc, in_maps, list(range(N_CORES)))
    out = np.zeros((B, S, D_MODEL), dtype=np.float32)
    for c in range(N_CORES):
        out[c // 4] += np.asarray(res.results[c]["y"], dtype=np.float32)
    return out
